# revision 12
# baseline (speedup 1.0000x reference)
"""TFEsmSelfAttention on 8 Trainium2 NeuronCores (Bass/Tile).

Sharding: core c -> batch b = c//4, heads 5*(c%4) .. 5*(c%4)+4.

Per-core pipeline (matmuls in float32r, ~2.6e-4 final rel err), with
projections and attention interleaved by s-quarter so the ScalarE exp
stream (the serial bottleneck, ~176us) starts ~35us in instead of after
the whole projection phase:

  for each s-quarter qq:
    K^T, Q^T projections (transposed layout [cols, s]; RoPE via
      partition-swap DMAs + fused scalar_tensor_tensor / gpsimd ops;
      W streamed from DRAM per quarter to fit SBUF),
    V projection (bias via K=1 ones-matmul; exp(mask) folded in as a
      per-row scale; an extra em column per head for softmax rowsums),
    attention bundle: every block (head-pair, q-group<=qq) processes the
      k-chunks that became available this quarter:
        scores^T chunk pairs via row-tiled K=64 matmuls (two heads
        concurrently in array row groups 0-63/64-127) -> PSUM span,
        exp on ScalarE (PSUM -> SBUF, f32r out),
        ctx^T accumulation with [V|em] as the 128x65 stationary
        (rowsum in partition 64), partial sums flushed to SBUF.
  finalize per block as its last chunk lands: reciprocal of rowsums,
    ones-broadcast matmul, normalize on DVE, DMA ctx^T out.

Host folds 1/sqrt(D) into Wq/bq, transposes X, exps the additive mask,
duplicates head 4 columns (uniform row-tiled pairing), and assembles the
final [B,S,HID] output from the per-core ctx^T results.
"""

import numpy as np

import concourse.bacc as bacc
import concourse.bass as bass
import concourse.mybir as mybir
import concourse.tile as tile

F32R = mybir.dt.float32r
F32 = mybir.dt.float32

B, S, HID = 2, 2048, 1280
H, D = 20, 64
HPC = 5
NCORES = 8
NCH = 10          # hid chunks of 128
NKC = 16          # k chunks of 128 (= s tiles)
NG = 4            # q groups (s quarters) of 512
QW = 512
PT = 3            # partition tiles of Q^T/K^T (col blocks h0,h1|h2,h3|h4,h4)

_PROGRAM = None


def _enable_ldw_opt():
    """walrus is invoked with --enable-ldw-opt=false; enabling it lets
    LDWEIGHTS overlap in-flight matmuls slightly better."""
    import concourse.bass_utils as bu

    if getattr(bu, "_ldw_opt_patched", False):
        return
    orig = bu.run_command

    def patched(argv, **kw):
        argv = [
            a.replace("--enable-ldw-opt=false", "--enable-ldw-opt=true")
            if isinstance(a, str) else a
            for a in argv
        ]
        return orig(argv, **kw)

    bu.run_command = patched
    bu._ldw_opt_patched = True


_enable_ldw_opt()


def _build_program():
    nc = bacc.Bacc("TRN2", target_bir_lowering=False, debug=False,
                   num_devices=NCORES)
    AF = mybir.AluOpType

    xt_d = nc.dram_tensor("xt", [HID, S], F32R, kind="ExternalInput")
    wq_d = nc.dram_tensor("wq", [HID, 384], F32R, kind="ExternalInput")
    wk_d = nc.dram_tensor("wk", [HID, 384], F32R, kind="ExternalInput")
    wv_d = nc.dram_tensor("wv", [HID, 320], F32R, kind="ExternalInput")
    bq_d = nc.dram_tensor("bq", [128, PT], F32, kind="ExternalInput")
    bk_d = nc.dram_tensor("bk", [128, PT], F32, kind="ExternalInput")
    bv_d = nc.dram_tensor("bv", [1, 320], F32R, kind="ExternalInput")
    ones1_d = nc.dram_tensor("ones1", [1, 128], F32R, kind="ExternalInput")
    ones64_d = nc.dram_tensor("ones64", [1, 64], F32R, kind="ExternalInput")
    cos_d = nc.dram_tensor("cosr", [128, S], F32, kind="ExternalInput")
    sin_d = nc.dram_tensor("sins", [128, S], F32, kind="ExternalInput")
    em_d = nc.dram_tensor("emask", [128, NKC], F32, kind="ExternalInput")
    out_d = nc.dram_tensor("ctxT", [320, S], F32, kind="ExternalOutput")

    with tile.TileContext(nc) as tc:
        with (
            tc.tile_pool(name="persist", bufs=1) as pp,
            tc.tile_pool(name="qpool", bufs=1) as qp,
            tc.tile_pool(name="cspool", bufs=1) as csp,
            tc.tile_pool(name="xtp", bufs=2) as xtp,
            tc.tile_pool(name="tabp", bufs=1) as tabp,
            tc.tile_pool(name="ropep", bufs=2) as rp,
            tc.tile_pool(name="ptp", bufs=3) as ptp,
            tc.tile_pool(name="finp", bufs=1) as fin_p,
            tc.tile_pool(name="mm1", bufs=2, space="PSUM") as mm1,
            tc.tile_pool(name="scsp", bufs=2, space="PSUM") as scsp,
        ):
            wk_sb = pp.tile([128, NCH, 384], F32R, name="wk_sb")
            wv_sb = pp.tile([128, NCH, 320], F32R, name="wv_sb")
            nc.sync.dma_start(wv_sb[:], wv_d.rearrange("(c p) n -> p c n", p=128))
            wq_sb = pp.tile([128, NCH, 384], F32R, name="wq_sb")
            nc.sync.dma_start(wk_sb[:], wk_d.rearrange("(c p) n -> p c n", p=128))
            nc.sync.dma_start(wq_sb[:], wq_d.rearrange("(c p) n -> p c n", p=128))
            k_sb = pp.tile([128, PT, S], F32R, name="k_sb")
            v_sb = pp.tile([128, NKC, 325], F32R, name="v_sb")
            bq_sb = pp.tile([128, PT], F32, name="bq_sb")
            bk_sb = pp.tile([128, PT], F32, name="bk_sb")
            bv_sb = pp.tile([1, 320], F32R, name="bv_sb")
            ones1_sb = pp.tile([1, 128], F32R, name="ones1_sb")
            ones64_sb = pp.tile([1, 64], F32R, name="ones64_sb")
            em_sb = pp.tile([128, NKC], F32, name="em_sb")

            nc.sync.dma_start(bq_sb[:], bq_d[:])
            nc.sync.dma_start(bk_sb[:], bk_d[:])
            nc.sync.dma_start(bv_sb[:], bv_d[:])
            nc.sync.dma_start(ones1_sb[:], ones1_d[:])
            nc.sync.dma_start(ones64_sb[:], ones64_d[:])
            nc.sync.dma_start(em_sb[:], em_d[:])

            def rope_block(ps, bias_sb, pt, cos_t, sin_t, out_ap):
                qraw = rp.tile([128, QW], F32, tag="qraw", name="qraw", bufs=1)
                nc.vector.tensor_scalar(
                    qraw[:], ps[:], bias_sb[:, pt : pt + 1], None, AF.add
                )
                t1 = rp.tile([128, QW], F32, tag="t1", name="t1", bufs=1)
                nc.vector.scalar_tensor_tensor(
                    t1[:], ps[:], bias_sb[:, pt : pt + 1], cos_t[:],
                    AF.add, AF.mult,
                )
                qsw = rp.tile([128, QW], F32, tag="qsw", name="qsw", bufs=1)
                for blk in range(4):
                    src = blk + 1 if blk % 2 == 0 else blk - 1
                    nc.sync.dma_start(
                        qsw[32 * blk : 32 * (blk + 1), :],
                        qraw[32 * src : 32 * (src + 1), :],
                    )
                t2 = rp.tile([128, QW], F32, tag="t2", name="t2", bufs=1)
                nc.gpsimd.tensor_tensor(t2[:], qsw[:], sin_t[:], AF.mult)
                nc.vector.tensor_tensor(out_ap, t1[:], t2[:], AF.add)

            q_tiles = {}
            cs_tiles = {}

            def finalize_block(g, heads_cs):
                """heads_cs: list of (head, cs_tile). Reciprocal + bcast +
                normalize + out DMA."""
                n = len(heads_cs)
                rtmp = fin_p.tile([2, QW], F32, tag="rtmp", name="rtmp")
                for j, (h, cs) in enumerate(heads_cs):
                    nc.sync.dma_start(rtmp[j : j + 1, :], cs[64:65, :])
                rr = fin_p.tile([2, QW], F32R, tag="rr", name="rr")
                with nc.allow_low_precision(reason="f32r recip for bcast mm"):
                    nc.vector.reciprocal(rr[0:n, :], rtmp[0:n, :])
                for j, (h, cs) in enumerate(heads_cs):
                    rst = fin_p.tile([1, QW], F32R, tag="rst", name="rst")
                    nc.sync.dma_start(rst[:], rr[j : j + 1, :])
                    rb = mm1.tile([64, QW], F32, tag="mm1", name=f"rb_{g}_{h}")
                    nc.tensor.matmul(rb[:], ones64_sb[:], rst[:],
                                     start=True, stop=True)
                    ob = fin_p.tile([64, QW], F32, tag="ob", name=f"ob_{g}_{h}")
                    nc.vector.tensor_tensor(ob[:], cs[0:64, :], rb[:], AF.mult)
                    nc.sync.dma_start(
                        out_d[64 * h : 64 * (h + 1), QW * g : QW * (g + 1)],
                        ob[:],
                    )

            for qq in range(NG):
                # ---- xt halves for this quarter ----
                xtA = xtp.tile([128, 5, QW], F32R, tag="xth", name=f"xtA_{qq}")
                xtB = xtp.tile([128, 5, QW], F32R, tag="xth", name=f"xtB_{qq}")
                xre = xt_d.rearrange("(c p) s -> p c s", p=128)
                nc.sync.dma_start(xtA[:], xre[:, 0:5, QW * qq : QW * (qq + 1)])
                nc.sync.dma_start(xtB[:], xre[:, 5:10, QW * qq : QW * (qq + 1)])

                def xsl(c, lo, hi):
                    t = xtA if c < 5 else xtB
                    return t[:, c % 5, lo:hi]

                cos_t = tabp.tile([128, QW], F32, tag="cost", name=f"cos_{qq}")
                sin_t = tabp.tile([128, QW], F32, tag="sint", name=f"sin_{qq}")
                nc.sync.dma_start(cos_t[:], cos_d[:, QW * qq : QW * (qq + 1)])
                nc.sync.dma_start(sin_t[:], sin_d[:, QW * qq : QW * (qq + 1)])

                # ---- K/Q projections (resident W) ----
                for t, pt in [("k", 0), ("k", 1), ("k", 2),
                              ("q", 0), ("q", 1), ("q", 2)]:
                    w_sb = wk_sb if t == "k" else wq_sb
                    ps = mm1.tile([128, QW], F32, tag="mm1",
                                  name=f"ps_{qq}_{t}{pt}")
                    for c in range(NCH):
                        nc.tensor.matmul(
                            ps[:],
                            w_sb[:, c, 128 * pt : 128 * (pt + 1)],
                            xsl(c, 0, QW),
                            start=(c == 0), stop=(c == NCH - 1),
                        )
                    if t == "k":
                        rope_block(ps, bk_sb, pt, cos_t, sin_t,
                                   k_sb[:, pt, QW * qq : QW * (qq + 1)])
                    else:
                        qt = qp.tile([128, QW], F32R, tag=f"qt_{qq}_{pt}",
                                     name=f"qt_{qq}_{pt}")
                        rope_block(ps, bq_sb, pt, cos_t, sin_t, qt[:])
                        q_tiles[(qq, pt)] = qt

                # ---- V waves ----
                for wv_i, (s0, s1) in enumerate([(0, 1), (2, 3)]):
                    ps0 = mm1.tile([128, QW], F32, tag="mm1",
                                   name=f"psv_{qq}_{s0}")
                    ps1 = mm1.tile([128, QW], F32, tag="mm1",
                                   name=f"psv_{qq}_{s1}")
                    for c in range(NCH):
                        nc.tensor.matmul(
                            ps0[:, 0:320], xsl(c, 128 * s0, 128 * (s0 + 1)),
                            wv_sb[:, c, :], start=(c == 0), stop=False,
                        )
                        nc.tensor.matmul(
                            ps1[:, 0:320], xsl(c, 128 * s1, 128 * (s1 + 1)),
                            wv_sb[:, c, :], start=(c == 0), stop=False,
                        )
                    for ps, stl in ((ps0, s0), (ps1, s1)):
                        st = 4 * qq + stl
                        nc.tensor.matmul(
                            ps[:, 0:320], ones1_sb[:], bv_sb[:],
                            start=False, stop=True,
                        )
                        vdst = v_sb[:, st, :].rearrange("p (h e) -> p h e", e=65)
                        nc.vector.tensor_scalar(
                            vdst[:, :, 0:64],
                            ps[:, 0:320].rearrange("p (h e) -> p h e", e=64),
                            em_sb[:, st : st + 1],
                            None,
                            AF.mult,
                        )
                        nc.vector.tensor_copy(
                            vdst[:, :, 64:65],
                            em_sb[:, st : st + 1]
                            .broadcast_to((128, HPC))
                            .rearrange("p (h e) -> p h e", e=1),
                        )

                # ---- attention bundle ----
                for g in range(qq + 1):
                    for hp in range(PT):
                        qt = q_tiles[(g, hp)]
                        c_lo = 4 * qq if g < qq else 0
                        c_hi = 4 * qq + 4
                        first_seg = g == qq
                        if hp < 2:
                            hA, hB = 2 * hp, 2 * hp + 1
                            ctxA = mm1.tile([65, QW], F32, tag="ctxA", bufs=1,
                                             name=f"ctxA_{qq}_{g}_{hp}")
                            ctxB = mm1.tile([65, QW], F32, tag="ctxB", bufs=1,
                                             name=f"ctxB_{qq}_{g}_{hp}")
                            pts = {}

                            def ctx_pair01(c):
                                nc.tensor.matmul(
                                    ctxA[:],
                                    v_sb[:, c, 65 * hA : 65 * (hA + 1)],
                                    pts[c][:, 0:512],
                                    start=(c == c_lo), stop=(c == c_hi - 1),
                                )
                                nc.tensor.matmul(
                                    ctxB[:],
                                    v_sb[:, c, 65 * hB : 65 * (hB + 1)],
                                    pts[c][:, 512:1024],
                                    start=(c == c_lo), stop=(c == c_hi - 1),
                                )

                            for c in range(c_lo, c_hi):
                                span = scsp.tile([128, 1024], F32, tag="scsp",
                                                 name=f"sc_{qq}_{g}_{hp}_{c}")
                                nc.tensor.matmul(
                                    span[:, 0:512],
                                    k_sb[0:64, hp, 128 * c : 128 * (c + 1)],
                                    qt[0:64, :],
                                    start=True, stop=True,
                                )
                                nc.tensor.matmul(
                                    span[:, 512:1024],
                                    k_sb[64:128, hp, 128 * c : 128 * (c + 1)],
                                    qt[64:128, :],
                                    start=True, stop=True,
                                    tile_position=(64, 0),
                                )
                                pt_t = ptp.tile([128, 1024], F32R, tag="pt",
                                                name=f"pt_{qq}_{g}_{hp}_{c}")
                                nc.scalar.activation(
                                    pt_t[:], span[:],
                                    mybir.ActivationFunctionType.Exp,
                                )
                                pts[c] = pt_t
                                if c - 2 >= c_lo:
                                    ctx_pair01(c - 2)
                            for c in range(max(c_hi - 2, c_lo), c_hi):
                                ctx_pair01(c)
                            segs = [(hA, ctxA), (hB, ctxB)]
                        else:
                            ctxA = mm1.tile([65, QW], F32, tag="ctxA", bufs=1,
                                             name=f"ctxA_{qq}_{g}_{hp}")
                            pts2 = {}

                            def ctx_pair2(s_):
                                c0, c1 = 2 * s_, 2 * s_ + 1
                                nc.tensor.matmul(
                                    ctxA[:],
                                    v_sb[:, c0, 65 * 4 : 65 * 5],
                                    pts2[s_][:, 0:512],
                                    start=(s_ == c_lo // 2), stop=False,
                                )
                                nc.tensor.matmul(
                                    ctxA[:],
                                    v_sb[:, c1, 65 * 4 : 65 * 5],
                                    pts2[s_][:, 512:1024],
                                    start=False, stop=(s_ == c_hi // 2 - 1),
                                )

                            for s_ in range(c_lo // 2, c_hi // 2):
                                c0, c1 = 2 * s_, 2 * s_ + 1
                                span = scsp.tile([128, 1024], F32, tag="scsp",
                                                 name=f"sc_{qq}_{g}_{hp}_{s_}")
                                nc.tensor.matmul(
                                    span[:, 0:512],
                                    k_sb[0:64, 2, 128 * c0 : 128 * (c0 + 1)],
                                    qt[0:64, :],
                                    start=True, stop=True,
                                )
                                nc.tensor.matmul(
                                    span[:, 512:1024],
                                    k_sb[64:128, 2, 128 * c1 : 128 * (c1 + 1)],
                                    qt[64:128, :],
                                    start=True, stop=True,
                                    tile_position=(64, 0),
                                )
                                pt_t = ptp.tile([128, 1024], F32R, tag="pt",
                                                name=f"pt_{qq}_{g}_{hp}_{s_}")
                                nc.scalar.activation(
                                    pt_t[:], span[:],
                                    mybir.ActivationFunctionType.Exp,
                                )
                                pts2[s_] = pt_t
                                if s_ - 2 >= c_lo // 2:
                                    ctx_pair2(s_ - 2)
                            for s_ in range(max(c_hi // 2 - 2, c_lo // 2), c_hi // 2):
                                ctx_pair2(s_)
                            segs = [(4, ctxA)]

                        done = []
                        for h, ctx_ps in segs:
                            if first_seg:
                                cs = csp.tile([65, QW], F32, tag=f"cs_{g}_{h}",
                                              name=f"cs_{g}_{h}")
                                nc.vector.tensor_copy(cs[:], ctx_ps[:])
                                cs_tiles[(g, h)] = cs
                            else:
                                cs = cs_tiles[(g, h)]
                                nc.vector.tensor_tensor(
                                    cs[:], cs[:], ctx_ps[:], AF.add
                                )
                            done.append((h, cs))
                        if qq == NG - 1:
                            finalize_block(g, done)

    nc.compile()
    return nc


def _host_inputs(hidden_states, attention_mask, Wq, bq, Wk, bk, Wv, bv):
    hs = np.asarray(hidden_states, np.float32)
    mask = np.asarray(attention_mask, np.float32).reshape(B, S)
    Wq = np.asarray(Wq, np.float32)
    Wk = np.asarray(Wk, np.float32)
    Wv = np.asarray(Wv, np.float32)
    bq = np.asarray(bq, np.float32)
    bk = np.asarray(bk, np.float32)
    bv = np.asarray(bv, np.float32)

    scale = float(D) ** -0.5

    i = np.arange(32)
    invf = 10000.0 ** (-i / 32.0)
    t = np.arange(S, dtype=np.float64)
    ang = t[None, :] * invf[:, None]
    cos32 = np.cos(ang).astype(np.float32)
    sin32 = np.sin(ang).astype(np.float32)
    cos64 = np.concatenate([cos32, cos32], 0)
    sin64 = np.concatenate([-sin32, sin32], 0)
    cosr = np.ascontiguousarray(np.concatenate([cos64, cos64], 0))
    sins = np.ascontiguousarray(np.concatenate([sin64, sin64], 0))

    ones1 = np.ones((1, 128), np.float32)
    ones64 = np.ones((1, 64), np.float32)

    in_maps = []
    for c in range(NCORES):
        b = c // 4
        h0 = HPC * (c % 4)
        heads = [h0, h0 + 1, h0 + 2, h0 + 3, h0 + 4, h0 + 4]
        colsq = np.concatenate([np.arange(64 * h, 64 * (h + 1)) for h in heads])
        colsv = colsq[: 64 * HPC]
        in_maps.append(
            {
                "xt": np.ascontiguousarray(hs[b].T),
                "wq": np.ascontiguousarray(Wq[:, colsq] * scale),
                "wk": np.ascontiguousarray(Wk[:, colsq]),
                "wv": np.ascontiguousarray(Wv[:, colsv]),
                "bq": np.ascontiguousarray((bq[colsq] * scale).reshape(PT, 128).T),
                "bk": np.ascontiguousarray(bk[colsq].reshape(PT, 128).T),
                "bv": np.ascontiguousarray(bv[colsv].reshape(1, 320)),
                "ones1": ones1,
                "ones64": ones64,
                "cosr": cosr,
                "sins": sins,
                "emask": np.ascontiguousarray(
                    np.exp(mask[b]).astype(np.float32).reshape(NKC, 128).T
                ),
            }
        )
    return in_maps


def kernel(hidden_states, attention_mask, Wq, bq, Wk, bk, Wv, bv):
    global _PROGRAM
    if _PROGRAM is None:
        _PROGRAM = _build_program()
    nc = _PROGRAM

    from concourse.bass_utils import run_bass_kernel_spmd

    in_maps = _host_inputs(hidden_states, attention_mask, Wq, bq, Wk, bk, Wv, bv)
    res = run_bass_kernel_spmd(nc, in_maps, list(range(NCORES)))

    out = np.empty((B, S, HID), np.float32)
    for c in range(NCORES):
        b = c // 4
        h0 = HPC * (c % 4)
        ctxT = res.results[c]["ctxT"]
        out[b, :, 64 * h0 : 64 * (h0 + HPC)] = ctxT.T
    return out


# revision 14
# speedup vs baseline: 1.2515x; 1.2515x over previous
"""TFEsmSelfAttention on 8 Trainium2 NeuronCores (Bass/Tile).

Sharding: core c -> batch b = c//4, heads 5*(c%4) .. 5*(c%4)+4.

Per-core pipeline (matmuls in float32r, ~2.6e-4 final rel err), with
projections and attention interleaved by s-quarter so the ScalarE exp
stream (the serial bottleneck, ~176us) starts ~35us in instead of after
the whole projection phase:

  for each s-quarter qq:
    K^T, Q^T projections (transposed layout [cols, s]; RoPE via
      partition-swap DMAs + fused scalar_tensor_tensor / gpsimd ops;
      W streamed from DRAM per quarter to fit SBUF),
    V projection (bias via K=1 ones-matmul; exp(mask) folded in as a
      per-row scale; an extra em column per head for softmax rowsums),
    attention bundle: every block (head-pair, q-group<=qq) processes the
      k-chunks that became available this quarter:
        scores^T chunk pairs via row-tiled K=64 matmuls (two heads
        concurrently in array row groups 0-63/64-127) -> PSUM span,
        exp on ScalarE (PSUM -> SBUF, f32r out),
        ctx^T accumulation with [V|em] as the 128x65 stationary
        (rowsum in partition 64), partial sums flushed to SBUF.
  finalize per block as its last chunk lands: reciprocal of rowsums,
    ones-broadcast matmul, normalize on DVE, DMA ctx^T out.

Host folds 1/sqrt(D) into Wq/bq, transposes X, exps the additive mask,
duplicates head 4 columns (uniform row-tiled pairing), and assembles the
final [B,S,HID] output from the per-core ctx^T results.
"""

import numpy as np

import concourse.bacc as bacc
import concourse.bass as bass
import concourse.mybir as mybir
import concourse.tile as tile

F32R = mybir.dt.float32r
F32 = mybir.dt.float32

B, S, HID = 2, 2048, 1280
H, D = 20, 64
HPC = 5
NCORES = 8
NCH = 10          # hid chunks of 128
NKC = 16          # k chunks of 128 (= s tiles)
NG = 4            # q groups (s quarters) of 512
QW = 512
PT = 3            # partition tiles of Q^T/K^T (col blocks h0,h1|h2,h3|h4,h4)

_PROGRAM = None


def _enable_ldw_opt():
    """walrus is invoked with --enable-ldw-opt=false; enabling it lets
    LDWEIGHTS overlap in-flight matmuls slightly better."""
    import concourse.bass_utils as bu

    if getattr(bu, "_ldw_opt_patched", False):
        return
    orig = bu.run_command

    def patched(argv, **kw):
        argv = [
            a.replace("--enable-ldw-opt=false", "--enable-ldw-opt=true")
            if isinstance(a, str) else a
            for a in argv
        ]
        return orig(argv, **kw)

    bu.run_command = patched
    bu._ldw_opt_patched = True


_enable_ldw_opt()


def _build_program():
    nc = bacc.Bacc("TRN2", target_bir_lowering=False, debug=False,
                   num_devices=NCORES)
    AF = mybir.AluOpType

    xt_d = nc.dram_tensor("xt", [HID, S], F32R, kind="ExternalInput")
    wq_d = nc.dram_tensor("wq", [HID, 384], F32R, kind="ExternalInput")
    wk_d = nc.dram_tensor("wk", [HID, 384], F32R, kind="ExternalInput")
    wv_d = nc.dram_tensor("wv", [HID, 320], F32R, kind="ExternalInput")
    bq_d = nc.dram_tensor("bq", [128, PT], F32, kind="ExternalInput")
    bk_d = nc.dram_tensor("bk", [128, PT], F32, kind="ExternalInput")
    bv_d = nc.dram_tensor("bv", [1, 320], F32R, kind="ExternalInput")
    ones1_d = nc.dram_tensor("ones1", [1, 128], F32R, kind="ExternalInput")
    ones64_d = nc.dram_tensor("ones64", [1, 64], F32R, kind="ExternalInput")
    cos_d = nc.dram_tensor("cosr", [128, S], F32, kind="ExternalInput")
    sin_d = nc.dram_tensor("sins", [128, S], F32, kind="ExternalInput")
    em_d = nc.dram_tensor("emask", [128, NKC], F32, kind="ExternalInput")
    out_d = nc.dram_tensor("ctxT", [320, S], F32, kind="ExternalOutput")

    with tile.TileContext(nc) as tc:
        with (
            tc.tile_pool(name="persist", bufs=1) as pp,
            tc.tile_pool(name="qpool", bufs=1) as qp,
            tc.tile_pool(name="cspool", bufs=1) as csp,
            tc.tile_pool(name="xtp", bufs=2) as xtp,
            tc.tile_pool(name="tabp", bufs=1) as tabp,
            tc.tile_pool(name="ropep", bufs=2) as rp,
            tc.tile_pool(name="ptp", bufs=3) as ptp,
            tc.tile_pool(name="finp", bufs=1) as fin_p,
            tc.tile_pool(name="mm1", bufs=2, space="PSUM") as mm1,
            tc.tile_pool(name="scsp", bufs=2, space="PSUM") as scsp,
        ):
            wk_sb = pp.tile([128, NCH, 384], F32R, name="wk_sb")
            wv_sb = pp.tile([128, NCH, 320], F32R, name="wv_sb")
            nc.sync.dma_start(wv_sb[:], wv_d.rearrange("(c p) n -> p c n", p=128))
            wq_sb = pp.tile([128, NCH, 384], F32R, name="wq_sb")
            nc.sync.dma_start(wk_sb[:], wk_d.rearrange("(c p) n -> p c n", p=128))
            nc.sync.dma_start(wq_sb[:], wq_d.rearrange("(c p) n -> p c n", p=128))
            k_sb = pp.tile([128, PT, S], F32R, name="k_sb")
            v_sb = pp.tile([128, NKC, 325], F32R, name="v_sb")
            bq_sb = pp.tile([128, PT], F32, name="bq_sb")
            bk_sb = pp.tile([128, PT], F32, name="bk_sb")
            bv_sb = pp.tile([1, 320], F32R, name="bv_sb")
            ones1_sb = pp.tile([1, 128], F32R, name="ones1_sb")
            ones64_sb = pp.tile([1, 64], F32R, name="ones64_sb")
            em_sb = pp.tile([128, NKC], F32, name="em_sb")

            nc.sync.dma_start(bq_sb[:], bq_d[:])
            nc.sync.dma_start(bk_sb[:], bk_d[:])
            nc.sync.dma_start(bv_sb[:], bv_d[:])
            nc.sync.dma_start(ones1_sb[:], ones1_d[:])
            nc.sync.dma_start(ones64_sb[:], ones64_d[:])
            nc.sync.dma_start(em_sb[:], em_d[:])

            def rope_block(ps, bias_sb, pt, cos_t, sin_t, out_ap):
                qraw = rp.tile([128, QW], F32, tag="qraw", name="qraw", bufs=1)
                nc.vector.tensor_scalar(
                    qraw[:], ps[:], bias_sb[:, pt : pt + 1], None, AF.add
                )
                t1 = rp.tile([128, QW], F32, tag="t1", name="t1", bufs=1)
                nc.vector.scalar_tensor_tensor(
                    t1[:], ps[:], bias_sb[:, pt : pt + 1], cos_t[:],
                    AF.add, AF.mult,
                )
                qsw = rp.tile([128, QW], F32, tag="qsw", name="qsw", bufs=1)
                for blk in range(4):
                    src = blk + 1 if blk % 2 == 0 else blk - 1
                    nc.sync.dma_start(
                        qsw[32 * blk : 32 * (blk + 1), :],
                        qraw[32 * src : 32 * (src + 1), :],
                    )
                t2 = rp.tile([128, QW], F32, tag="t2", name="t2", bufs=1)
                nc.gpsimd.tensor_tensor(t2[:], qsw[:], sin_t[:], AF.mult)
                nc.vector.tensor_tensor(out_ap, t1[:], t2[:], AF.add)

            q_tiles = {}
            cs_tiles = {}

            pending = {"gen": None, "left": 0}

            def drain(frac_units_left):
                gen = pending["gen"]
                if gen is None:
                    return
                n = max(1, -(-pending["left"] // max(frac_units_left, 1)))
                took = 0
                try:
                    while took < n:
                        next(gen)
                        took += 1
                except StopIteration:
                    pending["gen"] = None
                pending["left"] -= took

            def bundle_steps(qq):
                """Generator: one scores-span step per next(); trailing ctx."""
                AFL = mybir.AluOpType
                for g in range(qq + 1):
                    for hp in range(PT):
                        qt = q_tiles[(g, hp)]
                        c_lo = 4 * qq if g < qq else 0
                        c_hi = 4 * qq + 4
                        first_seg = g == qq
                        if hp < 2:
                            hA, hB = 2 * hp, 2 * hp + 1
                            ctxA = mm1.tile([65, QW], F32, tag="ctxA", bufs=1,
                                            name=f"ctxA_{qq}_{g}_{hp}")
                            ctxB = mm1.tile([65, QW], F32, tag="ctxB", bufs=1,
                                            name=f"ctxB_{qq}_{g}_{hp}")
                            pts = {}

                            def ctx_pair01(c, ctxA=ctxA, ctxB=ctxB, pts=pts,
                                           c_lo=c_lo, c_hi=c_hi, hA=hA, hB=hB):
                                nc.tensor.matmul(
                                    ctxA[:],
                                    v_sb[:, c, 65 * hA : 65 * (hA + 1)],
                                    pts[c][:, 0:512],
                                    start=(c == c_lo), stop=(c == c_hi - 1),
                                )
                                nc.tensor.matmul(
                                    ctxB[:],
                                    v_sb[:, c, 65 * hB : 65 * (hB + 1)],
                                    pts[c][:, 512:1024],
                                    start=(c == c_lo), stop=(c == c_hi - 1),
                                )

                            for c in range(c_lo, c_hi):
                                span = scsp.tile([128, 1024], F32, tag="scsp",
                                                 name=f"sc_{qq}_{g}_{hp}_{c}")
                                nc.tensor.matmul(
                                    span[:, 0:512],
                                    k_sb[0:64, hp, 128 * c : 128 * (c + 1)],
                                    qt[0:64, :],
                                    start=True, stop=True,
                                )
                                nc.tensor.matmul(
                                    span[:, 512:1024],
                                    k_sb[64:128, hp, 128 * c : 128 * (c + 1)],
                                    qt[64:128, :],
                                    start=True, stop=True,
                                    tile_position=(64, 0),
                                )
                                pt_t = ptp.tile([128, 1024], F32R, tag="pt",
                                                name=f"pt_{qq}_{g}_{hp}_{c}")
                                nc.scalar.activation(
                                    pt_t[:], span[:],
                                    mybir.ActivationFunctionType.Exp,
                                )
                                pts[c] = pt_t
                                if c - 2 >= c_lo:
                                    ctx_pair01(c - 2)
                                yield
                            for c in range(max(c_hi - 2, c_lo), c_hi):
                                ctx_pair01(c)
                            segs = [(hA, ctxA), (hB, ctxB)]
                        else:
                            ctxA = mm1.tile([65, QW], F32, tag="ctxA", bufs=1,
                                            name=f"ctxA_{qq}_{g}_{hp}")
                            pts2 = {}

                            def ctx_pair2(s_, ctxA=ctxA, pts2=pts2,
                                          c_lo=c_lo, c_hi=c_hi):
                                c0, c1 = 2 * s_, 2 * s_ + 1
                                nc.tensor.matmul(
                                    ctxA[:],
                                    v_sb[:, c0, 65 * 4 : 65 * 5],
                                    pts2[s_][:, 0:512],
                                    start=(s_ == c_lo // 2), stop=False,
                                )
                                nc.tensor.matmul(
                                    ctxA[:],
                                    v_sb[:, c1, 65 * 4 : 65 * 5],
                                    pts2[s_][:, 512:1024],
                                    start=False, stop=(s_ == c_hi // 2 - 1),
                                )

                            for s_ in range(c_lo // 2, c_hi // 2):
                                c0, c1 = 2 * s_, 2 * s_ + 1
                                span = scsp.tile([128, 1024], F32, tag="scsp",
                                                 name=f"sc_{qq}_{g}_{hp}_{s_}")
                                nc.tensor.matmul(
                                    span[:, 0:512],
                                    k_sb[0:64, 2, 128 * c0 : 128 * (c0 + 1)],
                                    qt[0:64, :],
                                    start=True, stop=True,
                                )
                                nc.tensor.matmul(
                                    span[:, 512:1024],
                                    k_sb[64:128, 2, 128 * c1 : 128 * (c1 + 1)],
                                    qt[64:128, :],
                                    start=True, stop=True,
                                    tile_position=(64, 0),
                                )
                                pt_t = ptp.tile([128, 1024], F32R, tag="pt",
                                                name=f"pt_{qq}_{g}_{hp}_{s_}")
                                nc.scalar.activation(
                                    pt_t[:], span[:],
                                    mybir.ActivationFunctionType.Exp,
                                )
                                pts2[s_] = pt_t
                                if s_ - 2 >= c_lo // 2:
                                    ctx_pair2(s_ - 2)
                                yield
                            for s_ in range(max(c_hi // 2 - 2, c_lo // 2),
                                            c_hi // 2):
                                ctx_pair2(s_)
                            segs = [(4, ctxA)]

                        done = []
                        for h, ctx_ps in segs:
                            if first_seg:
                                cs = csp.tile([65, QW], F32, tag=f"cs_{g}_{h}",
                                              name=f"cs_{g}_{h}")
                                nc.vector.tensor_copy(cs[:], ctx_ps[:])
                                cs_tiles[(g, h)] = cs
                            else:
                                cs = cs_tiles[(g, h)]
                                nc.vector.tensor_tensor(
                                    cs[:], cs[:], ctx_ps[:], AFL.add
                                )
                            done.append((h, cs))
                        if qq == NG - 1:
                            finalize_block(g, done)


            def finalize_block(g, heads_cs):
                """heads_cs: list of (head, cs_tile). Reciprocal + bcast +
                normalize + out DMA."""
                n = len(heads_cs)
                rtmp = fin_p.tile([2, QW], F32, tag="rtmp", name="rtmp")
                for j, (h, cs) in enumerate(heads_cs):
                    nc.sync.dma_start(rtmp[j : j + 1, :], cs[64:65, :])
                rr = fin_p.tile([2, QW], F32R, tag="rr", name="rr")
                with nc.allow_low_precision(reason="f32r recip for bcast mm"):
                    nc.vector.reciprocal(rr[0:n, :], rtmp[0:n, :])
                for j, (h, cs) in enumerate(heads_cs):
                    rst = fin_p.tile([1, QW], F32R, tag="rst", name="rst")
                    nc.sync.dma_start(rst[:], rr[j : j + 1, :])
                    rb = mm1.tile([64, QW], F32, tag="mm1", name=f"rb_{g}_{h}")
                    nc.tensor.matmul(rb[:], ones64_sb[:], rst[:],
                                     start=True, stop=True)
                    ob = fin_p.tile([64, QW], F32, tag="ob", name=f"ob_{g}_{h}")
                    nc.vector.tensor_tensor(ob[:], cs[0:64, :], rb[:], AF.mult)
                    nc.sync.dma_start(
                        out_d[64 * h : 64 * (h + 1), QW * g : QW * (g + 1)],
                        ob[:],
                    )

            for qq in range(NG):
                wi_unit = [0]
                # ---- xt halves for this quarter ----
                xtA = xtp.tile([128, 5, QW], F32R, tag="xth", name=f"xtA_{qq}")
                xtB = xtp.tile([128, 5, QW], F32R, tag="xth", name=f"xtB_{qq}")
                xre = xt_d.rearrange("(c p) s -> p c s", p=128)
                nc.sync.dma_start(xtA[:], xre[:, 0:5, QW * qq : QW * (qq + 1)])
                nc.sync.dma_start(xtB[:], xre[:, 5:10, QW * qq : QW * (qq + 1)])

                def xsl(c, lo, hi):
                    t = xtA if c < 5 else xtB
                    return t[:, c % 5, lo:hi]

                cos_t = tabp.tile([128, QW], F32, tag="cost", name=f"cos_{qq}")
                sin_t = tabp.tile([128, QW], F32, tag="sint", name=f"sin_{qq}")
                nc.sync.dma_start(cos_t[:], cos_d[:, QW * qq : QW * (qq + 1)])
                nc.sync.dma_start(sin_t[:], sin_d[:, QW * qq : QW * (qq + 1)])

                # ---- K/Q projections (resident W) ----
                for t, pt in [("k", 0), ("k", 1), ("k", 2),
                              ("q", 0), ("q", 1), ("q", 2)]:
                    w_sb = wk_sb if t == "k" else wq_sb
                    ps = mm1.tile([128, QW], F32, tag="mm1",
                                  name=f"ps_{qq}_{t}{pt}")
                    for c in range(NCH):
                        nc.tensor.matmul(
                            ps[:],
                            w_sb[:, c, 128 * pt : 128 * (pt + 1)],
                            xsl(c, 0, QW),
                            start=(c == 0), stop=(c == NCH - 1),
                        )
                    if t == "k":
                        rope_block(ps, bk_sb, pt, cos_t, sin_t,
                                   k_sb[:, pt, QW * qq : QW * (qq + 1)])
                    else:
                        qt = qp.tile([128, QW], F32R, tag=f"qt_{qq}_{pt}",
                                     name=f"qt_{qq}_{pt}")
                        rope_block(ps, bq_sb, pt, cos_t, sin_t, qt[:])
                        q_tiles[(qq, pt)] = qt
                    units_left = (5 - wi_unit[0]) + 2
                    drain(units_left)
                    wi_unit[0] += 1

                # ---- V waves ----
                for wv_i, (s0, s1) in enumerate([(0, 1), (2, 3)]):
                    ps0 = mm1.tile([128, QW], F32, tag="mm1",
                                   name=f"psv_{qq}_{s0}")
                    ps1 = mm1.tile([128, QW], F32, tag="mm1",
                                   name=f"psv_{qq}_{s1}")
                    for c in range(NCH):
                        nc.tensor.matmul(
                            ps0[:, 0:320], xsl(c, 128 * s0, 128 * (s0 + 1)),
                            wv_sb[:, c, :], start=(c == 0), stop=False,
                        )
                        nc.tensor.matmul(
                            ps1[:, 0:320], xsl(c, 128 * s1, 128 * (s1 + 1)),
                            wv_sb[:, c, :], start=(c == 0), stop=False,
                        )
                    for ps, stl in ((ps0, s0), (ps1, s1)):
                        st = 4 * qq + stl
                        nc.tensor.matmul(
                            ps[:, 0:320], ones1_sb[:], bv_sb[:],
                            start=False, stop=True,
                        )
                        vdst = v_sb[:, st, :].rearrange("p (h e) -> p h e", e=65)
                        nc.vector.tensor_scalar(
                            vdst[:, :, 0:64],
                            ps[:, 0:320].rearrange("p (h e) -> p h e", e=64),
                            em_sb[:, st : st + 1],
                            None,
                            AF.mult,
                        )
                        nc.vector.tensor_copy(
                            vdst[:, :, 64:65],
                            em_sb[:, st : st + 1]
                            .broadcast_to((128, HPC))
                            .rearrange("p (h e) -> p h e", e=1),
                        )

                # finish interleaving the previous bundle, then queue this one
                while pending["gen"] is not None:
                    drain(1)
                pending["gen"] = bundle_steps(qq)
                pending["left"] = (2 * qq + 1) * 10
                if qq == NG - 1:
                    while pending["gen"] is not None:
                        drain(1)

    nc.compile()
    return nc


def _host_inputs(hidden_states, attention_mask, Wq, bq, Wk, bk, Wv, bv):
    hs = np.asarray(hidden_states, np.float32)
    mask = np.asarray(attention_mask, np.float32).reshape(B, S)
    Wq = np.asarray(Wq, np.float32)
    Wk = np.asarray(Wk, np.float32)
    Wv = np.asarray(Wv, np.float32)
    bq = np.asarray(bq, np.float32)
    bk = np.asarray(bk, np.float32)
    bv = np.asarray(bv, np.float32)

    scale = float(D) ** -0.5

    i = np.arange(32)
    invf = 10000.0 ** (-i / 32.0)
    t = np.arange(S, dtype=np.float64)
    ang = t[None, :] * invf[:, None]
    cos32 = np.cos(ang).astype(np.float32)
    sin32 = np.sin(ang).astype(np.float32)
    cos64 = np.concatenate([cos32, cos32], 0)
    sin64 = np.concatenate([-sin32, sin32], 0)
    cosr = np.ascontiguousarray(np.concatenate([cos64, cos64], 0))
    sins = np.ascontiguousarray(np.concatenate([sin64, sin64], 0))

    ones1 = np.ones((1, 128), np.float32)
    ones64 = np.ones((1, 64), np.float32)

    in_maps = []
    for c in range(NCORES):
        b = c // 4
        h0 = HPC * (c % 4)
        heads = [h0, h0 + 1, h0 + 2, h0 + 3, h0 + 4, h0 + 4]
        colsq = np.concatenate([np.arange(64 * h, 64 * (h + 1)) for h in heads])
        colsv = colsq[: 64 * HPC]
        in_maps.append(
            {
                "xt": np.ascontiguousarray(hs[b].T),
                "wq": np.ascontiguousarray(Wq[:, colsq] * scale),
                "wk": np.ascontiguousarray(Wk[:, colsq]),
                "wv": np.ascontiguousarray(Wv[:, colsv]),
                "bq": np.ascontiguousarray((bq[colsq] * scale).reshape(PT, 128).T),
                "bk": np.ascontiguousarray(bk[colsq].reshape(PT, 128).T),
                "bv": np.ascontiguousarray(bv[colsv].reshape(1, 320)),
                "ones1": ones1,
                "ones64": ones64,
                "cosr": cosr,
                "sins": sins,
                "emask": np.ascontiguousarray(
                    np.exp(mask[b]).astype(np.float32).reshape(NKC, 128).T
                ),
            }
        )
    return in_maps


def kernel(hidden_states, attention_mask, Wq, bq, Wk, bk, Wv, bv):
    global _PROGRAM
    if _PROGRAM is None:
        _PROGRAM = _build_program()
    nc = _PROGRAM

    from concourse.bass_utils import run_bass_kernel_spmd

    in_maps = _host_inputs(hidden_states, attention_mask, Wq, bq, Wk, bk, Wv, bv)
    res = run_bass_kernel_spmd(nc, in_maps, list(range(NCORES)))

    out = np.empty((B, S, HID), np.float32)
    for c in range(NCORES):
        b = c // 4
        h0 = HPC * (c % 4)
        ctxT = res.results[c]["ctxT"]
        out[b, :, 64 * h0 : 64 * (h0 + HPC)] = ctxT.T
    return out
